# revision 1
# baseline (speedup 1.0000x reference)
"""Trainium2 Bass kernel for nn_NodeModel (GNN message passing).

Reference computation:
    h   = relu(concat(x[row], edge_attr) @ W1 + b1) @ W2 + b2     # edge MLP
    agg = scatter_mean(h, col, N)                                  # per-dest mean
    out = relu(concat(x, agg) @ W3 + b3) @ W4 + b4                 # node MLP

Distribution strategy (8 cores, no collectives needed):
  - Sort edges by destination node; split destination nodes into 8
    block-aligned, edge-balanced shards.  Each core owns one node shard and
    ALL edges targeting it, so per-node sums are complete locally.
  - x is replicated; each core gathers x[row] for its edges with indirect
    DMA on-device.
  - Edge MLP runs with weights stationary and activations kept transposed
    [feat, edge]; h2 rows are staged to DRAM.
  - Scatter-mean per 128-node block: indirect-gather the block's h2 rows,
    build a one-hot selection matrix with is_equal against an iota, and
    matmul-accumulate S^T @ h2 in PSUM; scale by 1/count.
  - Node MLP on the local shard; outputs are concatenated on host.

All matmuls run in float32r (TF32-like, full PE rate); accumulation fp32.
"""

import math
import sys
from contextlib import ExitStack

sys.path.insert(0, "/opt/trn_rl_repo")

import numpy as np

import concourse.bass as bass
import concourse.tile as tile
from concourse import bacc, mybir
from concourse.bass_utils import run_bass_kernel_spmd

NCORES = 8
P = 128
FN = 512    # node feature dim
FE = 128    # edge feature dim
HID = 1280  # edge-MLP hidden/output dim
F32 = mybir.dt.float32
F32R = mybir.dt.float32r
I32 = mybir.dt.int32
RELU = mybir.ActivationFunctionType.Relu

_prog_cache = {}


def _build(EC, NB, KB, NX):
    """Build the SPMD program for one core.

    EC: edge chunks (128 edges each) per core, multiple of 4.
    NB: node blocks (128 nodes each) per core, multiple of 4.
    KB: max edge chunks per node block (scatter schedule width).
    NX: number of rows of the replicated x (gather source).
    """
    EP = EC * P
    NBP = NB * P
    SC = EC // 4   # superchunks of 512 edges
    NSB = NB // 4  # superblocks of 512 nodes

    nc = bacc.Bacc("TRN2", target_bir_lowering=False, debug=False,
                   num_devices=NCORES)

    x_d = nc.dram_tensor("x", [NX, FN], F32R, kind="ExternalInput")
    rows_d = nc.dram_tensor("rows", [P, EC], I32, kind="ExternalInput")
    eaT_d = nc.dram_tensor("eaT", [FE, EP], F32R, kind="ExternalInput")
    W1_d = nc.dram_tensor("W1", [FN + FE, HID], F32R, kind="ExternalInput")
    W2_d = nc.dram_tensor("W2", [HID, HID], F32R, kind="ExternalInput")
    W3_d = nc.dram_tensor("W3", [FN + HID, FN + FE], F32R, kind="ExternalInput")
    W4_d = nc.dram_tensor("W4", [FN + FE, FN], F32R, kind="ExternalInput")
    b1_d = nc.dram_tensor("b1", [P, HID // P], F32, kind="ExternalInput")
    b2_d = nc.dram_tensor("b2", [P, HID // P], F32, kind="ExternalInput")
    b3_d = nc.dram_tensor("b3", [P, (FN + FE) // P], F32, kind="ExternalInput")
    b4_d = nc.dram_tensor("b4", [P, FN // P], F32, kind="ExternalInput")
    gid_d = nc.dram_tensor("gid", [P, NB * KB], I32, kind="ExternalInput")
    colb_d = nc.dram_tensor("colb", [P, NB * KB], F32, kind="ExternalInput")
    invc_d = nc.dram_tensor("invc", [P, NB], F32, kind="ExternalInput")
    xsT_d = nc.dram_tensor("xsT", [FN, NBP], F32R, kind="ExternalInput")
    iota_d = nc.dram_tensor("iota", [P, P], F32, kind="ExternalInput")
    ident_d = nc.dram_tensor("ident", [P, P], F32R, kind="ExternalInput")
    out_d = nc.dram_tensor("out", [NBP, FN], F32, kind="ExternalOutput")
    h2_d = nc.dram_tensor("h2buf", [EP, HID], F32R)  # internal staging

    with tile.TileContext(nc) as tc, ExitStack() as ctx:
        cpool = ctx.enter_context(tc.tile_pool(name="const", bufs=1))
        ptp = ctx.enter_context(tc.tile_pool(name="ptp", bufs=2, space="PSUM"))

        identt = cpool.tile([P, P], F32R)
        nc.sync.dma_start(identt[:], ident_d.ap()[:])
        iotat = cpool.tile([P, P], F32)
        nc.sync.dma_start(iotat[:], iota_d.ap()[:])
        b1t = cpool.tile([P, HID // P], F32)
        nc.sync.dma_start(b1t[:], b1_d.ap()[:])
        b2t = cpool.tile([P, HID // P], F32)
        nc.sync.dma_start(b2t[:], b2_d.ap()[:])
        b3t = cpool.tile([P, (FN + FE) // P], F32)
        nc.sync.dma_start(b3t[:], b3_d.ap()[:])
        b4t = cpool.tile([P, FN // P], F32)
        nc.sync.dma_start(b4t[:], b4_d.ap()[:])
        rowst = cpool.tile([P, EC], I32)
        nc.sync.dma_start(rowst[:], rows_d.ap()[:])
        gidt = cpool.tile([P, NB * KB], I32)
        nc.sync.dma_start(gidt[:], gid_d.ap()[:])
        colbt = cpool.tile([P, NB * KB], F32)
        nc.sync.dma_start(colbt[:], colb_d.ap()[:])
        invct = cpool.tile([P, NB], F32)
        nc.sync.dma_start(invct[:], invc_d.ap()[:])

        # ---------------- Phase E: edge MLP ----------------
        with ExitStack() as ectx:
            wpool = ectx.enter_context(tc.tile_pool(name="wE", bufs=1))
            W1t = wpool.tile([P, 5, HID], F32R)
            nc.sync.dma_start(
                W1t[:], W1_d.ap().rearrange("(ko ki) m -> ki ko m", ki=P))
            W2t = wpool.tile([P, 10, HID], F32R)
            nc.sync.dma_start(
                W2t[:], W2_d.ap().rearrange("(ko ki) m -> ki ko m", ki=P))

            xgp = ectx.enter_context(tc.tile_pool(name="xg", bufs=2))
            xgTp = ectx.enter_context(tc.tile_pool(name="xgT", bufs=2))
            eap = ectx.enter_context(tc.tile_pool(name="ea", bufs=2))
            h1p = ectx.enter_context(tc.tile_pool(name="h1T", bufs=1))
            h2Tp = ectx.enter_context(tc.tile_pool(name="h2T", bufs=1))
            h2op = ectx.enter_context(tc.tile_pool(name="h2o", bufs=4))
            mmp = ectx.enter_context(
                tc.tile_pool(name="mmE", bufs=4, space="PSUM"))

            for sc in range(SC):
                xgt = xgp.tile([P, 4, FN], F32R)
                for k in range(4):
                    nc.gpsimd.indirect_dma_start(
                        out=xgt[:, k, :], out_offset=None, in_=x_d.ap()[:],
                        in_offset=bass.IndirectOffsetOnAxis(
                            ap=rowst[:, sc * 4 + k:sc * 4 + k + 1], axis=0))
                xgTt = xgTp.tile([P, 4, 512], F32R)
                for f in range(4):
                    for k in range(4):
                        pt = ptp.tile([P, P], F32R)
                        nc.tensor.transpose(
                            pt[:], xgt[:, k, f * P:(f + 1) * P], identt[:])
                        nc.vector.tensor_copy(
                            xgTt[:, f, k * P:(k + 1) * P], pt[:])
                eat = eap.tile([P, 512], F32R)
                nc.sync.dma_start(
                    eat[:], eaT_d.ap()[:, sc * 512:(sc + 1) * 512])

                h1Tt = h1p.tile([P, 10, 512], F32R)
                for of in range(10):
                    ps = mmp.tile([P, 512], F32)
                    for k in range(5):
                        rhs = xgTt[:, k, :] if k < 4 else eat[:]
                        nc.tensor.matmul(
                            ps[:], W1t[:, k, of * P:(of + 1) * P], rhs,
                            start=(k == 0), stop=(k == 4))
                    nc.scalar.activation(h1Tt[:, of, :], ps[:], RELU,
                                         bias=b1t[:, of:of + 1])
                h2Tt = h2Tp.tile([P, 10, 512], F32R)
                for of in range(10):
                    ps = mmp.tile([P, 512], F32)
                    for k in range(10):
                        nc.tensor.matmul(
                            ps[:], W2t[:, k, of * P:(of + 1) * P],
                            h1Tt[:, k, :], start=(k == 0), stop=(k == 9))
                    nc.vector.tensor_scalar_add(h2Tt[:, of, :], ps[:],
                                                b2t[:, of:of + 1])
                for k in range(4):
                    h2ot = h2op.tile([P, HID], F32R)
                    for of in range(10):
                        pt = ptp.tile([P, P], F32R)
                        nc.tensor.transpose(
                            pt[:], h2Tt[:, of, k * P:(k + 1) * P], identt[:])
                        nc.vector.tensor_copy(
                            h2ot[:, of * P:(of + 1) * P], pt[:])
                    r0 = sc * 512 + k * P
                    nc.sync.dma_start(h2_d.ap()[r0:r0 + P, :], h2ot[:])

        # ---------------- Phases S+N: scatter-mean + node MLP ----------------
        with ExitStack() as sctx:
            wpool2 = sctx.enter_context(tc.tile_pool(name="wN", bufs=1))
            W3t = wpool2.tile([P, 14, FN + FE], F32R)
            nc.sync.dma_start(
                W3t[:], W3_d.ap().rearrange("(ko ki) m -> ki ko m", ki=P))
            W4t = wpool2.tile([P, 5, FN], F32R)
            nc.sync.dma_start(
                W4t[:], W4_d.ap().rearrange("(ko ki) m -> ki ko m", ki=P))

            h2gp = sctx.enter_context(tc.tile_pool(name="h2g", bufs=3))
            Sp = sctx.enter_context(tc.tile_pool(name="Smat", bufs=3))
            aggp = sctx.enter_context(tc.tile_pool(name="agg", bufs=2))
            aggTp = sctx.enter_context(tc.tile_pool(name="aggT", bufs=2))
            xsp = sctx.enter_context(tc.tile_pool(name="xs", bufs=2))
            h3p = sctx.enter_context(tc.tile_pool(name="h3T", bufs=2))
            oTp = sctx.enter_context(tc.tile_pool(name="oT", bufs=2))
            ogp = sctx.enter_context(tc.tile_pool(name="og", bufs=4))
            smp = sctx.enter_context(
                tc.tile_pool(name="smp", bufs=1, space="PSUM"))
            mmp2 = sctx.enter_context(
                tc.tile_pool(name="mmN", bufs=2, space="PSUM"))

            nj = (HID + 511) // 512  # psum 512-slices of the scatter output

            for s in range(NSB):
                aggTt = aggTp.tile([P, 10, 512], F32R)
                for bb in range(4):
                    b = s * 4 + bb
                    pss = smp.tile([P, HID], F32)
                    for k in range(KB):
                        c = b * KB + k
                        h2g = h2gp.tile([P, HID], F32R)
                        nc.gpsimd.indirect_dma_start(
                            out=h2g[:], out_offset=None, in_=h2_d.ap()[:],
                            in_offset=bass.IndirectOffsetOnAxis(
                                ap=gidt[:, c:c + 1], axis=0))
                        St = Sp.tile([P, P], F32R)
                        nc.vector.tensor_tensor(
                            St[:], colbt[:, c:c + 1].to_broadcast([P, P]),
                            iotat[:], op=mybir.AluOpType.is_equal)
                        for j in range(nj):
                            lo, hi = j * 512, min((j + 1) * 512, HID)
                            nc.tensor.matmul(
                                pss[:, lo:hi], St[:], h2g[:, lo:hi],
                                start=(k == 0), stop=(k == KB - 1))
                    agg = aggp.tile([P, HID], F32R)
                    nc.vector.tensor_scalar_mul(agg[:], pss[:],
                                                invct[:, b:b + 1])
                    for f in range(10):
                        pt = ptp.tile([P, P], F32R)
                        nc.tensor.transpose(
                            pt[:], agg[:, f * P:(f + 1) * P], identt[:])
                        nc.vector.tensor_copy(
                            aggTt[:, f, bb * P:(bb + 1) * P], pt[:])

                xst = xsp.tile([P, 4, 512], F32R)
                nc.sync.dma_start(
                    xst[:],
                    xsT_d.ap().rearrange("(fo fi) n -> fi fo n", fi=P)
                    [:, :, s * 512:(s + 1) * 512])
                h3Tt = h3p.tile([P, 5, 512], F32R)
                for of in range(5):
                    ps = mmp2.tile([P, 512], F32)
                    for k in range(14):
                        rhs = xst[:, k, :] if k < 4 else aggTt[:, k - 4, :]
                        nc.tensor.matmul(
                            ps[:], W3t[:, k, of * P:(of + 1) * P], rhs,
                            start=(k == 0), stop=(k == 13))
                    nc.scalar.activation(h3Tt[:, of, :], ps[:], RELU,
                                         bias=b3t[:, of:of + 1])
                oTt = oTp.tile([P, 4, 512], F32R)
                for of in range(4):
                    ps = mmp2.tile([P, 512], F32)
                    for k in range(5):
                        nc.tensor.matmul(
                            ps[:], W4t[:, k, of * P:(of + 1) * P],
                            h3Tt[:, k, :], start=(k == 0), stop=(k == 4))
                    nc.vector.tensor_scalar_add(oTt[:, of, :], ps[:],
                                                b4t[:, of:of + 1])
                for g in range(4):
                    og = ogp.tile([P, FN], F32)
                    for of in range(4):
                        pt = ptp.tile([P, P], F32R)
                        nc.tensor.transpose(
                            pt[:], oTt[:, of, g * P:(g + 1) * P], identt[:])
                        nc.vector.tensor_copy(
                            og[:, of * P:(of + 1) * P],
                            pt[:].bitcast(F32))
                    r0 = s * 512 + g * P
                    nc.sync.dma_start(out_d.ap()[r0:r0 + P, :], og[:])

    nc.compile()
    return nc


def _prepare(x, row, col, ea):
    """Host-side sharding: sort edges by destination, split nodes into 8
    block-aligned edge-balanced shards, build per-core arrays."""
    N = x.shape[0]
    E = ea.shape[0]
    order = np.argsort(col, kind="stable")
    scol = col[order]
    srow = row[order]
    NBLK = (N + P - 1) // P
    NTOT = NBLK * P

    bounds = [0]
    for p in range(1, NCORES):
        if E > 0:
            t = int(scol[min((p * E) // NCORES, E - 1)])
        else:
            t = (p * NTOT) // NCORES
        b = int(round(t / P)) * P
        b = max(b, bounds[-1] + P)
        b = min(b, NTOT - P * (NCORES - p))
        bounds.append(b)
    bounds.append(NTOT)
    for p in range(1, NCORES + 1):
        assert bounds[p] > bounds[p - 1], f"degenerate shard bounds {bounds}"

    e_split = np.searchsorted(scol, bounds)
    Ec = np.diff(e_split)
    EC = max(4, math.ceil(int(Ec.max()) / P))
    EC = ((EC + 3) // 4) * 4
    EP = EC * P
    nblk = [(bounds[p + 1] - bounds[p]) // P for p in range(NCORES)]
    NB = max(4, ((max(nblk) + 3) // 4) * 4)
    NBP = NB * P
    blkdeg = np.bincount(scol // P, minlength=NBLK)
    KB = max(1, math.ceil(int(blkdeg.max()) / P))

    xpadT = np.zeros((FN, NTOT + NBP), np.float32)
    xpadT[:, :N] = x.T

    cores = []
    for p in range(NCORES):
        s, e = int(e_split[p]), int(e_split[p + 1])
        n0 = bounds[p]
        ne = e - s
        tmp = np.zeros(EP, np.int32)
        tmp[:ne] = srow[s:e]
        rows_t = np.ascontiguousarray(tmp.reshape(EC, P).T)
        eaT = np.zeros((FE, EP), np.float32)
        eaT[:, :ne] = ea[order[s:e]].T
        lcol = (scol[s:e] - n0).astype(np.int64)
        bstart = np.searchsorted(lcol, np.arange(NB + 1) * P)
        gid = np.zeros((NB, KB, P), np.int32)
        colb = np.full((NB, KB, P), -1.0, np.float32)
        for b in range(NB):
            sb, eb = int(bstart[b]), int(bstart[b + 1])
            cnt = eb - sb
            assert cnt <= KB * P
            gid[b].reshape(-1)[:cnt] = np.arange(sb, eb, dtype=np.int32)
            colb[b].reshape(-1)[:cnt] = (lcol[sb:eb] - b * P)
        gid_t = np.ascontiguousarray(gid.reshape(NB * KB, P).T)
        colb_t = np.ascontiguousarray(colb.reshape(NB * KB, P).T)
        deg = np.bincount(lcol, minlength=NBP)[:NBP]
        invc_t = np.ascontiguousarray(
            (1.0 / np.maximum(deg, 1.0)).astype(np.float32).reshape(NB, P).T)
        xsT = np.ascontiguousarray(xpadT[:, n0:n0 + NBP])
        cores.append(dict(rows=rows_t, eaT=eaT, gid=gid_t, colb=colb_t,
                          invc=invc_t, xsT=xsT))
    return cores, bounds, EC, NB, KB


def _run(inputs, trace=False):
    x = np.ascontiguousarray(np.asarray(inputs["x"], dtype=np.float32))
    ei = np.asarray(inputs["edge_index"])
    ea = np.ascontiguousarray(np.asarray(inputs["edge_attr"], dtype=np.float32))
    row = ei[0].astype(np.int64)
    col = ei[1].astype(np.int64)
    W1 = np.ascontiguousarray(np.asarray(inputs["W1"], np.float32))
    W2 = np.ascontiguousarray(np.asarray(inputs["W2"], np.float32))
    W3 = np.ascontiguousarray(np.asarray(inputs["W3"], np.float32))
    W4 = np.ascontiguousarray(np.asarray(inputs["W4"], np.float32))
    b1 = np.asarray(inputs["b1"], np.float32)
    b2 = np.asarray(inputs["b2"], np.float32)
    b3 = np.asarray(inputs["b3"], np.float32)
    b4 = np.asarray(inputs["b4"], np.float32)
    N = x.shape[0]

    cores, bounds, EC, NB, KB = _prepare(x, row, col, ea)

    key = (EC, NB, KB, N)
    if key not in _prog_cache:
        _prog_cache[key] = _build(EC, NB, KB, N)
    nc = _prog_cache[key]

    b1t = np.ascontiguousarray(b1.reshape(HID // P, P).T)
    b2t = np.ascontiguousarray(b2.reshape(HID // P, P).T)
    b3t = np.ascontiguousarray(b3.reshape((FN + FE) // P, P).T)
    b4t = np.ascontiguousarray(b4.reshape(FN // P, P).T)
    iota = np.ascontiguousarray(
        np.broadcast_to(np.arange(P, dtype=np.float32), (P, P)))
    ident = np.eye(P, dtype=np.float32)

    in_maps = []
    for p in range(NCORES):
        c = cores[p]
        in_maps.append({
            "x": x, "rows": c["rows"], "eaT": c["eaT"],
            "W1": W1, "W2": W2, "W3": W3, "W4": W4,
            "b1": b1t, "b2": b2t, "b3": b3t, "b4": b4t,
            "gid": c["gid"], "colb": c["colb"], "invc": c["invc"],
            "xsT": c["xsT"], "iota": iota, "ident": ident,
        })

    res = run_bass_kernel_spmd(nc, in_maps, list(range(NCORES)), trace=trace)

    out = np.empty((N, FN), np.float32)
    for p in range(NCORES):
        n0, n1 = bounds[p], min(bounds[p + 1], N)
        if n1 > n0:
            out[n0:n1] = res.results[p]["out"][:n1 - n0]
    return out, res


def kernel(**inputs) -> np.ndarray:
    out, _ = _run(inputs, trace=False)
    return out


# revision 6
# speedup vs baseline: 1.2093x; 1.2093x over previous
"""Trainium2 Bass kernel for nn_NodeModel (GNN message passing).

Reference computation:
    h   = relu(concat(x[row], edge_attr) @ W1 + b1) @ W2 + b2     # edge MLP
    agg = scatter_mean(h, col, N)                                  # per-dest mean
    out = relu(concat(x, agg) @ W3 + b3) @ W4 + b4                 # node MLP

Distribution strategy (8 cores, no collectives needed):
  - Sort edges by destination node; split destination nodes into 8
    block-aligned, edge-balanced shards.  Each core owns one node shard and
    ALL edges targeting it, so per-node sums are complete locally.
  - x is replicated; each core gathers x[row] for its edges with indirect
    DMA on-device.
  - Edge MLP runs with weights stationary and activations kept transposed
    [feat, edge]; h2 rows are staged to DRAM.
  - Scatter-mean per 128-node block: indirect-gather the block's h2 rows,
    build a one-hot selection matrix with is_equal against an iota, and
    matmul-accumulate S^T @ h2 in PSUM; scale by 1/count.
  - Node MLP on the local shard; outputs are concatenated on host.

All matmuls run in float32r (TF32-like, full PE rate); accumulation fp32.
"""

import math
import sys
from contextlib import ExitStack

sys.path.insert(0, "/opt/trn_rl_repo")

import numpy as np

import concourse.bass as bass
import concourse.tile as tile
from concourse import bacc, mybir
from concourse.bass_utils import run_bass_kernel_spmd

NCORES = 8
P = 128
FN = 512    # node feature dim
FE = 128    # edge feature dim
HID = 1280  # edge-MLP hidden/output dim
F32 = mybir.dt.float32
F32R = mybir.dt.float32r
I32 = mybir.dt.int32
RELU = mybir.ActivationFunctionType.Relu

_prog_cache = {}


def _build(EC, NB, KB, NX):
    """Build the SPMD program for one core.

    EC: edge chunks (128 edges each) per core, multiple of 4.
    NB: node blocks (128 nodes each) per core, multiple of 4.
    KB: max edge chunks per node block (scatter schedule width).
    NX: number of rows of the replicated x (gather source).
    """
    EP = EC * P
    NBP = NB * P
    SC = EC // 4   # superchunks of 512 edges
    NSB = NB // 4  # superblocks of 512 nodes

    nc = bacc.Bacc("TRN2", target_bir_lowering=False, debug=False,
                   num_devices=NCORES)

    x_d = nc.dram_tensor("x", [NX, FN], F32R, kind="ExternalInput")
    rows_d = nc.dram_tensor("rows", [P, EC], I32, kind="ExternalInput")
    eaT_d = nc.dram_tensor("eaT", [FE, EP], F32R, kind="ExternalInput")
    W1_d = nc.dram_tensor("W1", [FN + FE, HID], F32R, kind="ExternalInput")
    W2_d = nc.dram_tensor("W2", [HID, HID], F32R, kind="ExternalInput")
    W3_d = nc.dram_tensor("W3", [FN + HID, FN + FE], F32R, kind="ExternalInput")
    W4_d = nc.dram_tensor("W4", [FN + FE, FN], F32R, kind="ExternalInput")
    b1_d = nc.dram_tensor("b1", [P, HID // P], F32, kind="ExternalInput")
    b2_d = nc.dram_tensor("b2", [P, HID // P], F32, kind="ExternalInput")
    b3_d = nc.dram_tensor("b3", [P, (FN + FE) // P], F32, kind="ExternalInput")
    b4_d = nc.dram_tensor("b4", [P, FN // P], F32, kind="ExternalInput")
    gid_d = nc.dram_tensor("gid", [P, NB * KB], I32, kind="ExternalInput")
    colb_d = nc.dram_tensor("colb", [P, NB * KB], F32, kind="ExternalInput")
    invc_d = nc.dram_tensor("invc", [P, NB], F32, kind="ExternalInput")
    xsT_d = nc.dram_tensor("xsT", [FN, NBP], F32R, kind="ExternalInput")
    iota_d = nc.dram_tensor("iota", [P, P], F32, kind="ExternalInput")
    ident_d = nc.dram_tensor("ident", [P, P], F32R, kind="ExternalInput")
    out_d = nc.dram_tensor("out", [NBP, FN], F32, kind="ExternalOutput")
    h2_d = nc.dram_tensor("h2buf", [EP, HID], F32R)  # internal staging

    with tile.TileContext(nc) as tc, ExitStack() as ctx:
        cpool = ctx.enter_context(tc.tile_pool(name="const", bufs=1))
        ptp = ctx.enter_context(tc.tile_pool(name="ptp", bufs=2, space="PSUM"))

        identt = cpool.tile([P, P], F32R)
        nc.sync.dma_start(identt[:], ident_d.ap()[:])
        iotat = cpool.tile([P, P], F32)
        nc.sync.dma_start(iotat[:], iota_d.ap()[:])
        b1t = cpool.tile([P, HID // P], F32)
        nc.sync.dma_start(b1t[:], b1_d.ap()[:])
        b2t = cpool.tile([P, HID // P], F32)
        nc.sync.dma_start(b2t[:], b2_d.ap()[:])
        b3t = cpool.tile([P, (FN + FE) // P], F32)
        nc.sync.dma_start(b3t[:], b3_d.ap()[:])
        b4t = cpool.tile([P, FN // P], F32)
        nc.sync.dma_start(b4t[:], b4_d.ap()[:])
        rowst = cpool.tile([P, EC], I32)
        nc.sync.dma_start(rowst[:], rows_d.ap()[:])
        gidt = cpool.tile([P, NB * KB], I32)
        nc.sync.dma_start(gidt[:], gid_d.ap()[:])
        colbt = cpool.tile([P, NB * KB], F32)
        nc.sync.dma_start(colbt[:], colb_d.ap()[:])
        invct = cpool.tile([P, NB], F32)
        nc.sync.dma_start(invct[:], invc_d.ap()[:])

        # ---------------- Phase E: edge MLP ----------------
        # Transposes run in PE transpose-mode, which does NOT count as
        # PE activity for the HAM clock gate: a burst of back-to-back
        # transposes >3.4us re-throttles the PE to 1.2 GHz.  All
        # transposes are therefore interleaved between matmul groups,
        # and gathers are pipelined one superchunk ahead.
        with ExitStack() as ectx:
            wpool = ectx.enter_context(tc.tile_pool(name="wE", bufs=1))
            W1t = wpool.tile([P, 5, HID], F32R)
            W1r = W1_d.ap().rearrange("(ko ki) m -> ki ko m", ki=P)
            for k in range(5):
                nc.sync.dma_start(W1t[:, k, :], W1r[:, k, :])
            W2t = wpool.tile([P, 10, HID], F32R)
            W2r = W2_d.ap().rearrange("(ko ki) m -> ki ko m", ki=P)
            for k in range(10):
                nc.sync.dma_start(W2t[:, k, :], W2r[:, k, :])

            xgp = ectx.enter_context(tc.tile_pool(name="xg", bufs=2))
            xgTp = ectx.enter_context(tc.tile_pool(name="xgT", bufs=2))
            eap = ectx.enter_context(tc.tile_pool(name="ea", bufs=2))
            h1p = ectx.enter_context(tc.tile_pool(name="h1T", bufs=1))
            h2Tp = ectx.enter_context(tc.tile_pool(name="h2T", bufs=1))
            h2op = ectx.enter_context(tc.tile_pool(name="h2o", bufs=4))
            mmp = ectx.enter_context(
                tc.tile_pool(name="mmE", bufs=4, space="PSUM"))

            def issue_gather(sc):
                xgt = xgp.tile([P, 4, FN], F32R)
                for k in range(4):
                    nc.gpsimd.indirect_dma_start(
                        out=xgt[:, k, :], out_offset=None, in_=x_d.ap()[:],
                        in_offset=bass.IndirectOffsetOnAxis(
                            ap=rowst[:, sc * 4 + k:sc * 4 + k + 1], axis=0))
                eat = eap.tile([P, 512], F32R)
                nc.sync.dma_start(
                    eat[:], eaT_d.ap()[:, sc * 512:(sc + 1) * 512])
                return xgt, eat

            def entry_T(xgt, xgTt, f, k):
                pt = ptp.tile([P, P], F32R)
                nc.tensor.transpose(
                    pt[:], xgt[:, k, f * P:(f + 1) * P], identt[:])
                nc.vector.tensor_copy(xgTt[:, f, k * P:(k + 1) * P], pt[:])

            # prologue: superchunk 0 input + its entry transposes
            xg_cur, ea_cur = issue_gather(0)
            xgT_cur = xgTp.tile([P, 4, 512], F32R)
            for f in range(4):
                for k in range(4):
                    entry_T(xg_cur, xgT_cur, f, k)

            for sc in range(SC):
                if sc + 1 < SC:
                    xg_next, ea_next = issue_gather(sc + 1)
                    xgT_next = xgTp.tile([P, 4, 512], F32R)
                else:
                    xg_next = ea_next = xgT_next = None

                h1Tt = h1p.tile([P, 10, 512], F32R)
                for of in range(10):
                    ps = mmp.tile([P, 512], F32)
                    for k in range(5):
                        rhs = xgT_cur[:, k, :] if k < 4 else ea_cur[:]
                        nc.tensor.matmul(
                            ps[:], W1t[:, k, of * P:(of + 1) * P], rhs,
                            start=(k == 0), stop=(k == 4))
                    nc.scalar.activation(h1Tt[:, of, :], ps[:], RELU,
                                         bias=b1t[:, of:of + 1])
                h2Tt = h2Tp.tile([P, 10, 512], F32R)
                h2ot = [h2op.tile([P, HID], F32R, name=f"h2o_{sc}_{k}", tag="h2o")
                         for k in range(4)]
                for of in range(10):
                    ps = mmp.tile([P, 512], F32)
                    for k in range(10):
                        nc.tensor.matmul(
                            ps[:], W2t[:, k, of * P:(of + 1) * P],
                            h1Tt[:, k, :], start=(k == 0), stop=(k == 9))
                    nc.vector.tensor_scalar_add(h2Tt[:, of, :], ps[:],
                                                b2t[:, of:of + 1])
                    # interleave: this of-chunk's exit transposes
                    for k in range(4):
                        pt = ptp.tile([P, P], F32R)
                        nc.tensor.transpose(
                            pt[:], h2Tt[:, of, k * P:(k + 1) * P], identt[:])
                        nc.vector.tensor_copy(
                            h2ot[k][:, of * P:(of + 1) * P], pt[:])
                    # interleave: next superchunk's entry transposes
                    if xgT_next is not None and of < 8:
                        for k in range(2):
                            entry_T(xg_next, xgT_next, of // 2, (of % 2) * 2 + k)
                for k in range(4):
                    r0 = sc * 512 + k * P
                    nc.sync.dma_start(h2_d.ap()[r0:r0 + P, :], h2ot[k][:])
                xg_cur, ea_cur, xgT_cur = xg_next, ea_next, xgT_next

        # ---------------- Phases S+N: scatter-mean + node MLP ----------------
        with ExitStack() as sctx:
            wpool2 = sctx.enter_context(tc.tile_pool(name="wN", bufs=1))
            W3t = wpool2.tile([P, 14, FN + FE], F32R)
            nc.sync.dma_start(
                W3t[:], W3_d.ap().rearrange("(ko ki) m -> ki ko m", ki=P))
            W4t = wpool2.tile([P, 5, FN], F32R)
            nc.sync.dma_start(
                W4t[:], W4_d.ap().rearrange("(ko ki) m -> ki ko m", ki=P))

            h2gp = sctx.enter_context(tc.tile_pool(name="h2g", bufs=3))
            Sp = sctx.enter_context(tc.tile_pool(name="Smat", bufs=3))
            aggp = sctx.enter_context(tc.tile_pool(name="agg", bufs=2))
            aggTp = sctx.enter_context(tc.tile_pool(name="aggT", bufs=2))
            xsp = sctx.enter_context(tc.tile_pool(name="xs", bufs=2))
            h3p = sctx.enter_context(tc.tile_pool(name="h3T", bufs=2))
            oTp = sctx.enter_context(tc.tile_pool(name="oT", bufs=2))
            ogp = sctx.enter_context(tc.tile_pool(name="og", bufs=4))
            smp = sctx.enter_context(
                tc.tile_pool(name="smp", bufs=1, space="PSUM"))
            mmp2 = sctx.enter_context(
                tc.tile_pool(name="mmN", bufs=2, space="PSUM"))

            nj = (HID + 511) // 512  # psum 512-slices of the scatter output

            for s in range(NSB):
                aggTt = aggTp.tile([P, 10, 512], F32R)
                # pending aggT transposes: (agg_tile, bb) emitted lazily so
                # they interleave with the next block's scatter matmuls
                pend = []

                def emit_aggT(n):
                    for _ in range(n):
                        if not pend:
                            return
                        agg, bb2, f = pend.pop(0)
                        pt = ptp.tile([P, P], F32R)
                        nc.tensor.transpose(
                            pt[:], agg[:, f * P:(f + 1) * P], identt[:])
                        nc.vector.tensor_copy(
                            aggTt[:, f, bb2 * P:(bb2 + 1) * P], pt[:])

                for bb in range(4):
                    b = s * 4 + bb
                    pss = smp.tile([P, HID], F32)
                    for k in range(KB):
                        c = b * KB + k
                        h2g = h2gp.tile([P, HID], F32R)
                        nc.gpsimd.indirect_dma_start(
                            out=h2g[:], out_offset=None, in_=h2_d.ap()[:],
                            in_offset=bass.IndirectOffsetOnAxis(
                                ap=gidt[:, c:c + 1], axis=0))
                        St = Sp.tile([P, P], F32R)
                        nc.vector.tensor_tensor(
                            St[:], colbt[:, c:c + 1].to_broadcast([P, P]),
                            iotat[:], op=mybir.AluOpType.is_equal)
                        for j in range(nj):
                            lo, hi = j * 512, min((j + 1) * 512, HID)
                            nc.tensor.matmul(
                                pss[:, lo:hi], St[:], h2g[:, lo:hi],
                                start=(k == 0), stop=(k == KB - 1))
                        q = 10 // KB
                        emit_aggT(10 - (KB - 1) * q if k == KB - 1 else q)
                    agg = aggp.tile([P, HID], F32R)
                    nc.vector.tensor_scalar_mul(agg[:], pss[:],
                                                invct[:, b:b + 1])
                    pend.extend((agg, bb, f) for f in range(10))

                xst = xsp.tile([P, 4, 512], F32R)
                nc.sync.dma_start(
                    xst[:],
                    xsT_d.ap().rearrange("(fo fi) n -> fi fo n", fi=P)
                    [:, :, s * 512:(s + 1) * 512])
                h3Tt = h3p.tile([P, 5, 512], F32R)
                for of in range(5):
                    ps = mmp2.tile([P, 512], F32)
                    for k in range(4):
                        nc.tensor.matmul(
                            ps[:], W3t[:, k, of * P:(of + 1) * P],
                            xst[:, k, :], start=(k == 0), stop=False)
                    for f in range(10):
                        emit_aggT(1)  # last block's transposes, just in time
                        nc.tensor.matmul(
                            ps[:], W3t[:, 4 + f, of * P:(of + 1) * P],
                            aggTt[:, f, :], start=False, stop=(f == 9))
                    nc.scalar.activation(h3Tt[:, of, :], ps[:], RELU,
                                         bias=b3t[:, of:of + 1])
                emit_aggT(100)  # drain any stragglers (non-standard KB)
                oTt = oTp.tile([P, 4, 512], F32R)
                ogs = [ogp.tile([P, FN], F32, name=f"og_{s}_{g}", tag="og")
                       for g in range(4)]
                for of in range(4):
                    ps = mmp2.tile([P, 512], F32)
                    for k in range(5):
                        nc.tensor.matmul(
                            ps[:], W4t[:, k, of * P:(of + 1) * P],
                            h3Tt[:, k, :], start=(k == 0), stop=(k == 4))
                    nc.vector.tensor_scalar_add(oTt[:, of, :], ps[:],
                                                b4t[:, of:of + 1])
                    for g in range(4):
                        pt = ptp.tile([P, P], F32R)
                        nc.tensor.transpose(
                            pt[:], oTt[:, of, g * P:(g + 1) * P], identt[:])
                        nc.vector.tensor_copy(
                            ogs[g][:, of * P:(of + 1) * P],
                            pt[:].bitcast(F32))
                for g in range(4):
                    r0 = s * 512 + g * P
                    nc.sync.dma_start(out_d.ap()[r0:r0 + P, :], ogs[g][:])

    nc.compile()
    return nc


def _prepare(x, row, col, ea):
    """Host-side sharding: sort edges by destination, split nodes into 8
    block-aligned edge-balanced shards, build per-core arrays."""
    N = x.shape[0]
    E = ea.shape[0]
    order = np.argsort(col, kind="stable")
    scol = col[order]
    srow = row[order]
    NBLK = (N + P - 1) // P
    NTOT = NBLK * P

    bounds = [0]
    for p in range(1, NCORES):
        if E > 0:
            t = int(scol[min((p * E) // NCORES, E - 1)])
        else:
            t = (p * NTOT) // NCORES
        b = int(round(t / P)) * P
        b = max(b, bounds[-1] + P)
        b = min(b, NTOT - P * (NCORES - p))
        bounds.append(b)
    bounds.append(NTOT)
    for p in range(1, NCORES + 1):
        assert bounds[p] > bounds[p - 1], f"degenerate shard bounds {bounds}"

    e_split = np.searchsorted(scol, bounds)
    Ec = np.diff(e_split)
    EC = max(4, math.ceil(int(Ec.max()) / P))
    EC = ((EC + 3) // 4) * 4
    EP = EC * P
    nblk = [(bounds[p + 1] - bounds[p]) // P for p in range(NCORES)]
    NB = max(4, ((max(nblk) + 3) // 4) * 4)
    NBP = NB * P
    blkdeg = np.bincount(scol // P, minlength=NBLK)
    KB = max(1, math.ceil(int(blkdeg.max()) / P))

    xpadT = np.zeros((FN, NTOT + NBP), np.float32)
    xpadT[:, :N] = x.T

    cores = []
    for p in range(NCORES):
        s, e = int(e_split[p]), int(e_split[p + 1])
        n0 = bounds[p]
        ne = e - s
        tmp = np.zeros(EP, np.int32)
        tmp[:ne] = srow[s:e]
        rows_t = np.ascontiguousarray(tmp.reshape(EC, P).T)
        eaT = np.zeros((FE, EP), np.float32)
        eaT[:, :ne] = ea[order[s:e]].T
        lcol = (scol[s:e] - n0).astype(np.int64)
        bstart = np.searchsorted(lcol, np.arange(NB + 1) * P)
        gid = np.zeros((NB, KB, P), np.int32)
        colb = np.full((NB, KB, P), -1.0, np.float32)
        for b in range(NB):
            sb, eb = int(bstart[b]), int(bstart[b + 1])
            cnt = eb - sb
            assert cnt <= KB * P
            gid[b].reshape(-1)[:cnt] = np.arange(sb, eb, dtype=np.int32)
            colb[b].reshape(-1)[:cnt] = (lcol[sb:eb] - b * P)
        gid_t = np.ascontiguousarray(gid.reshape(NB * KB, P).T)
        colb_t = np.ascontiguousarray(colb.reshape(NB * KB, P).T)
        deg = np.bincount(lcol, minlength=NBP)[:NBP]
        invc_t = np.ascontiguousarray(
            (1.0 / np.maximum(deg, 1.0)).astype(np.float32).reshape(NB, P).T)
        xsT = np.ascontiguousarray(xpadT[:, n0:n0 + NBP])
        cores.append(dict(rows=rows_t, eaT=eaT, gid=gid_t, colb=colb_t,
                          invc=invc_t, xsT=xsT))
    return cores, bounds, EC, NB, KB


def _run(inputs, trace=False):
    x = np.ascontiguousarray(np.asarray(inputs["x"], dtype=np.float32))
    ei = np.asarray(inputs["edge_index"])
    ea = np.ascontiguousarray(np.asarray(inputs["edge_attr"], dtype=np.float32))
    row = ei[0].astype(np.int64)
    col = ei[1].astype(np.int64)
    W1 = np.ascontiguousarray(np.asarray(inputs["W1"], np.float32))
    W2 = np.ascontiguousarray(np.asarray(inputs["W2"], np.float32))
    W3 = np.ascontiguousarray(np.asarray(inputs["W3"], np.float32))
    W4 = np.ascontiguousarray(np.asarray(inputs["W4"], np.float32))
    b1 = np.asarray(inputs["b1"], np.float32)
    b2 = np.asarray(inputs["b2"], np.float32)
    b3 = np.asarray(inputs["b3"], np.float32)
    b4 = np.asarray(inputs["b4"], np.float32)
    N = x.shape[0]

    cores, bounds, EC, NB, KB = _prepare(x, row, col, ea)

    key = (EC, NB, KB, N)
    if key not in _prog_cache:
        _prog_cache[key] = _build(EC, NB, KB, N)
    nc = _prog_cache[key]

    b1t = np.ascontiguousarray(b1.reshape(HID // P, P).T)
    b2t = np.ascontiguousarray(b2.reshape(HID // P, P).T)
    b3t = np.ascontiguousarray(b3.reshape((FN + FE) // P, P).T)
    b4t = np.ascontiguousarray(b4.reshape(FN // P, P).T)
    iota = np.ascontiguousarray(
        np.broadcast_to(np.arange(P, dtype=np.float32), (P, P)))
    ident = np.eye(P, dtype=np.float32)

    in_maps = []
    for p in range(NCORES):
        c = cores[p]
        in_maps.append({
            "x": x, "rows": c["rows"], "eaT": c["eaT"],
            "W1": W1, "W2": W2, "W3": W3, "W4": W4,
            "b1": b1t, "b2": b2t, "b3": b3t, "b4": b4t,
            "gid": c["gid"], "colb": c["colb"], "invc": c["invc"],
            "xsT": c["xsT"], "iota": iota, "ident": ident,
        })

    res = run_bass_kernel_spmd(nc, in_maps, list(range(NCORES)), trace=trace)

    out = np.empty((N, FN), np.float32)
    for p in range(NCORES):
        n0, n1 = bounds[p], min(bounds[p + 1], N)
        if n1 > n0:
            out[n0:n1] = res.results[p]["out"][:n1 - n0]
    return out, res


def kernel(**inputs) -> np.ndarray:
    out, _ = _run(inputs, trace=False)
    return out


# revision 9
# speedup vs baseline: 1.2427x; 1.0276x over previous
"""Trainium2 Bass kernel for nn_NodeModel (GNN message passing).

Reference computation:
    h   = relu(concat(x[row], edge_attr) @ W1 + b1) @ W2 + b2     # edge MLP
    agg = scatter_mean(h, col, N)                                  # per-dest mean
    out = relu(concat(x, agg) @ W3 + b3) @ W4 + b4                 # node MLP

Distribution strategy (8 cores, no collectives needed):
  - Sort edges by destination node; split destination nodes into 8
    block-aligned, edge-balanced shards.  Each core owns one node shard and
    ALL edges targeting it, so per-node sums are complete locally.
  - x is replicated; each core gathers x[row] for its edges with indirect
    DMA on-device.
  - Edge MLP runs with weights stationary and activations kept transposed
    [feat, edge]; h2 rows are staged to DRAM.
  - Scatter-mean per 128-node block: indirect-gather the block's h2 rows,
    build a one-hot selection matrix with is_equal against an iota, and
    matmul-accumulate S^T @ h2 in PSUM; scale by 1/count.
  - Node MLP on the local shard; outputs are concatenated on host.

All matmuls run in float32r (TF32-like, full PE rate); accumulation fp32.
"""

import math
import sys
from contextlib import ExitStack

sys.path.insert(0, "/opt/trn_rl_repo")

import numpy as np

import concourse.bass as bass
import concourse.tile as tile
from concourse import bacc, mybir
from concourse.bass_utils import run_bass_kernel_spmd

NCORES = 8
P = 128
FN = 512    # node feature dim
FE = 128    # edge feature dim
HID = 1280  # edge-MLP hidden/output dim
F32 = mybir.dt.float32
F32R = mybir.dt.float32r
I32 = mybir.dt.int32
RELU = mybir.ActivationFunctionType.Relu

_prog_cache = {}


def _build(EC, NB, KB, NX):
    """Build the SPMD program for one core.

    EC: edge chunks (128 edges each) per core, multiple of 4.
    NB: node blocks (128 nodes each) per core, multiple of 4.
    KB: max edge chunks per node block (scatter schedule width).
    NX: number of rows of the replicated x (gather source).
    """
    EP = EC * P
    NBP = NB * P
    SC = EC // 4   # superchunks of 512 edges
    NSB = NB // 4  # superblocks of 512 nodes

    nc = bacc.Bacc("TRN2", target_bir_lowering=False, debug=False,
                   num_devices=NCORES)

    x_d = nc.dram_tensor("x", [NX, FN], F32R, kind="ExternalInput")
    rows_d = nc.dram_tensor("rows", [P, EC], I32, kind="ExternalInput")
    eaT_d = nc.dram_tensor("eaT", [FE, EP], F32R, kind="ExternalInput")
    W1_d = nc.dram_tensor("W1", [FN + FE, HID], F32R, kind="ExternalInput")
    W2_d = nc.dram_tensor("W2", [HID, HID], F32R, kind="ExternalInput")
    W3_d = nc.dram_tensor("W3", [FN + HID, FN + FE], F32R, kind="ExternalInput")
    W4_d = nc.dram_tensor("W4", [FN + FE, FN], F32R, kind="ExternalInput")
    b1_d = nc.dram_tensor("b1", [P, HID // P], F32, kind="ExternalInput")
    b2_d = nc.dram_tensor("b2", [P, HID // P], F32, kind="ExternalInput")
    b3_d = nc.dram_tensor("b3", [P, (FN + FE) // P], F32, kind="ExternalInput")
    b4_d = nc.dram_tensor("b4", [P, FN // P], F32, kind="ExternalInput")
    gid_d = nc.dram_tensor("gid", [P, NB * KB], I32, kind="ExternalInput")
    colb_d = nc.dram_tensor("colb", [P, NB * KB], F32, kind="ExternalInput")
    invc_d = nc.dram_tensor("invc", [P, NB], F32, kind="ExternalInput")
    xsT_d = nc.dram_tensor("xsT", [FN, NBP], F32R, kind="ExternalInput")
    iota_d = nc.dram_tensor("iota", [P, P], F32, kind="ExternalInput")
    ident_d = nc.dram_tensor("ident", [P, P], F32R, kind="ExternalInput")
    out_d = nc.dram_tensor("out", [NBP, FN], F32, kind="ExternalOutput")
    h2_d = nc.dram_tensor("h2buf", [EP, HID], F32R)  # internal staging

    with tile.TileContext(nc) as tc, ExitStack() as ctx:
        cpool = ctx.enter_context(tc.tile_pool(name="const", bufs=1))
        ptp = ctx.enter_context(tc.tile_pool(name="ptp", bufs=2, space="PSUM"))

        identt = cpool.tile([P, P], F32R)
        nc.sync.dma_start(identt[:], ident_d.ap()[:])
        iotat = cpool.tile([P, P], F32)
        nc.sync.dma_start(iotat[:], iota_d.ap()[:])
        b1t = cpool.tile([P, HID // P], F32)
        nc.sync.dma_start(b1t[:], b1_d.ap()[:])
        b2t = cpool.tile([P, HID // P], F32)
        nc.sync.dma_start(b2t[:], b2_d.ap()[:])
        b3t = cpool.tile([P, (FN + FE) // P], F32)
        nc.sync.dma_start(b3t[:], b3_d.ap()[:])
        b4t = cpool.tile([P, FN // P], F32)
        nc.sync.dma_start(b4t[:], b4_d.ap()[:])
        rowst = cpool.tile([P, EC], I32)
        nc.sync.dma_start(rowst[:], rows_d.ap()[:])
        gidt = cpool.tile([P, NB * KB], I32)
        nc.sync.dma_start(gidt[:], gid_d.ap()[:])
        colbt = cpool.tile([P, NB * KB], F32)
        nc.sync.dma_start(colbt[:], colb_d.ap()[:])
        invct = cpool.tile([P, NB], F32)
        nc.sync.dma_start(invct[:], invc_d.ap()[:])

        # ---------------- Phase E: edge MLP ----------------
        # Transposes run in PE transpose-mode, which does NOT count as
        # PE activity for the HAM clock gate: a burst of back-to-back
        # transposes >3.4us re-throttles the PE to 1.2 GHz.  All
        # transposes are therefore interleaved between matmul groups,
        # and gathers are pipelined one superchunk ahead.
        with ExitStack() as ectx:
            wpool = ectx.enter_context(tc.tile_pool(name="wE", bufs=1))
            W1t = wpool.tile([P, 5, HID], F32R)
            W1r = W1_d.ap().rearrange("(ko ki) m -> ki ko m", ki=P)
            for k in range(5):
                nc.sync.dma_start(W1t[:, k, :], W1r[:, k, :])
            W2t = wpool.tile([P, 10, HID], F32R)
            W2r = W2_d.ap().rearrange("(ko ki) m -> ki ko m", ki=P)
            for k in range(10):
                nc.sync.dma_start(W2t[:, k, :], W2r[:, k, :])

            xgp = ectx.enter_context(tc.tile_pool(name="xg", bufs=2))
            xgTp = ectx.enter_context(tc.tile_pool(name="xgT", bufs=2))
            eap = ectx.enter_context(tc.tile_pool(name="ea", bufs=2))
            h1p = ectx.enter_context(tc.tile_pool(name="h1T", bufs=1))
            h2Tp = ectx.enter_context(tc.tile_pool(name="h2T", bufs=1))
            h2op = ectx.enter_context(tc.tile_pool(name="h2o", bufs=4))
            mmp = ectx.enter_context(
                tc.tile_pool(name="mmE", bufs=4, space="PSUM"))

            def issue_gather(sc):
                xgt = xgp.tile([P, 4, FN], F32R)
                for k in range(4):
                    nc.gpsimd.indirect_dma_start(
                        out=xgt[:, k, :], out_offset=None, in_=x_d.ap()[:],
                        in_offset=bass.IndirectOffsetOnAxis(
                            ap=rowst[:, sc * 4 + k:sc * 4 + k + 1], axis=0))
                eat = eap.tile([P, 512], F32R)
                nc.sync.dma_start(
                    eat[:], eaT_d.ap()[:, sc * 512:(sc + 1) * 512])
                return xgt, eat

            def entry_T(xgt, xgTt, f, k):
                pt = ptp.tile([P, P], F32R)
                nc.tensor.transpose(
                    pt[:], xgt[:, k, f * P:(f + 1) * P], identt[:])
                nc.vector.tensor_copy(xgTt[:, f, k * P:(k + 1) * P], pt[:])

            # prologue: superchunk 0 input + its entry transposes
            xg_cur, ea_cur = issue_gather(0)
            xgT_cur = xgTp.tile([P, 4, 512], F32R)
            for f in range(4):
                for k in range(4):
                    entry_T(xg_cur, xgT_cur, f, k)

            for sc in range(SC):
                if sc + 1 < SC:
                    xg_next, ea_next = issue_gather(sc + 1)
                    xgT_next = xgTp.tile([P, 4, 512], F32R)
                else:
                    xg_next = ea_next = xgT_next = None

                h1Tt = h1p.tile([P, 10, 512], F32R)
                for of in range(10):
                    ps = mmp.tile([P, 512], F32)
                    for k in range(5):
                        rhs = xgT_cur[:, k, :] if k < 4 else ea_cur[:]
                        nc.tensor.matmul(
                            ps[:], W1t[:, k, of * P:(of + 1) * P], rhs,
                            start=(k == 0), stop=(k == 4))
                    nc.scalar.activation(h1Tt[:, of, :], ps[:], RELU,
                                         bias=b1t[:, of:of + 1])
                h2Tt = h2Tp.tile([P, 10, 512], F32R)
                h2ot = [h2op.tile([P, HID], F32R, name=f"h2o_{sc}_{k}", tag="h2o")
                         for k in range(4)]
                for of in range(10):
                    ps = mmp.tile([P, 512], F32)
                    for k in range(10):
                        nc.tensor.matmul(
                            ps[:], W2t[:, k, of * P:(of + 1) * P],
                            h1Tt[:, k, :], start=(k == 0), stop=(k == 9))
                    nc.vector.tensor_scalar_add(h2Tt[:, of, :], ps[:],
                                                b2t[:, of:of + 1])
                    # interleave: this of-chunk's exit transposes
                    for k in range(4):
                        pt = ptp.tile([P, P], F32R)
                        nc.tensor.transpose(
                            pt[:], h2Tt[:, of, k * P:(k + 1) * P], identt[:])
                        nc.vector.tensor_copy(
                            h2ot[k][:, of * P:(of + 1) * P], pt[:])
                    # interleave: next superchunk's entry transposes
                    if xgT_next is not None and of < 8:
                        for k in range(2):
                            entry_T(xg_next, xgT_next, of // 2, (of % 2) * 2 + k)
                for k in range(4):
                    r0 = sc * 512 + k * P
                    nc.sync.dma_start(h2_d.ap()[r0:r0 + P, :], h2ot[k][:])
                xg_cur, ea_cur, xgT_cur = xg_next, ea_next, xgT_next

        # ---------------- Phases S+N: scatter-mean + node MLP ----------------
        with ExitStack() as sctx:
            wpool2 = sctx.enter_context(tc.tile_pool(name="wN", bufs=1))
            W3t = wpool2.tile([P, 14, FN + FE], F32R)
            nc.sync.dma_start(
                W3t[:], W3_d.ap().rearrange("(ko ki) m -> ki ko m", ki=P))
            W4t = wpool2.tile([P, 5, FN], F32R)
            nc.sync.dma_start(
                W4t[:], W4_d.ap().rearrange("(ko ki) m -> ki ko m", ki=P))

            h2gp = sctx.enter_context(tc.tile_pool(name="h2g", bufs=7))
            Sp = sctx.enter_context(tc.tile_pool(name="Smat", bufs=3))
            aggp = sctx.enter_context(tc.tile_pool(name="agg", bufs=2))
            aggTp = sctx.enter_context(tc.tile_pool(name="aggT", bufs=2))
            xsp = sctx.enter_context(tc.tile_pool(name="xs", bufs=2))
            h3p = sctx.enter_context(tc.tile_pool(name="h3T", bufs=1))
            oTp = sctx.enter_context(tc.tile_pool(name="oT", bufs=2))
            ogp = sctx.enter_context(tc.tile_pool(name="og", bufs=4))
            smp = sctx.enter_context(
                tc.tile_pool(name="smp", bufs=1, space="PSUM"))
            mmp2 = sctx.enter_context(
                tc.tile_pool(name="mmN", bufs=2, space="PSUM"))

            nj = (HID + 511) // 512  # psum 512-slices of the scatter output

            # Rolling gather lookahead: block b's h2-row gathers (slow,
            # gpsimd SW-DGE) are issued one block ahead of its scatter
            # matmuls so the PE never waits on them.  Pad slots carry an
            # out-of-bounds id and are silently skipped by the DMA
            # (bounds_check), so padding costs no gather bandwidth.
            pend_gs = {}

            def gather_S(b):
                lst = []
                for k in range(KB):
                    c = b * KB + k
                    h2g = h2gp.tile([P, HID], F32R, name=f"h2g_{b}_{k}",
                                    tag="h2g")
                    nc.gpsimd.indirect_dma_start(
                        out=h2g[:], out_offset=None, in_=h2_d.ap()[:],
                        in_offset=bass.IndirectOffsetOnAxis(
                            ap=gidt[:, c:c + 1], axis=0),
                        bounds_check=EP - 1, oob_is_err=False)
                    St = Sp.tile([P, P], F32R, name=f"S_{b}_{k}", tag="S")
                    nc.vector.tensor_tensor(
                        St[:], colbt[:, c:c + 1].to_broadcast([P, P]),
                        iotat[:], op=mybir.AluOpType.is_equal)
                    lst.append((h2g, St))
                pend_gs[b] = lst

            gather_S(0)
            gather_S(1)

            for s in range(NSB):
                aggTt = aggTp.tile([P, 10, 512], F32R)
                # pending aggT transposes: (agg_tile, bb) emitted lazily so
                # they interleave with the next block's scatter matmuls
                pend = []

                def emit_aggT(n):
                    for _ in range(n):
                        if not pend:
                            return
                        agg, bb2, f = pend.pop(0)
                        pt = ptp.tile([P, P], F32R)
                        nc.tensor.transpose(
                            pt[:], agg[:, f * P:(f + 1) * P], identt[:])
                        nc.vector.tensor_copy(
                            aggTt[:, f, bb2 * P:(bb2 + 1) * P], pt[:])

                for bb in range(4):
                    b = s * 4 + bb
                    if b + 2 < NB:
                        gather_S(b + 2)
                    pss = smp.tile([P, HID], F32)
                    for k, (h2g, St) in enumerate(pend_gs.pop(b)):
                        for j in range(nj):
                            lo, hi = j * 512, min((j + 1) * 512, HID)
                            nc.tensor.matmul(
                                pss[:, lo:hi], St[:], h2g[:, lo:hi],
                                start=(k == 0), stop=(k == KB - 1))
                        q = 10 // KB
                        emit_aggT(10 - (KB - 1) * q if k == KB - 1 else q)
                    agg = aggp.tile([P, HID], F32R)
                    nc.vector.tensor_scalar_mul(agg[:], pss[:],
                                                invct[:, b:b + 1])
                    pend.extend((agg, bb, f) for f in range(10))

                xst = xsp.tile([P, 4, 512], F32R)
                nc.sync.dma_start(
                    xst[:],
                    xsT_d.ap().rearrange("(fo fi) n -> fi fo n", fi=P)
                    [:, :, s * 512:(s + 1) * 512])
                h3Tt = h3p.tile([P, 5, 512], F32R)
                for of in range(5):
                    ps = mmp2.tile([P, 512], F32)
                    for k in range(4):
                        nc.tensor.matmul(
                            ps[:], W3t[:, k, of * P:(of + 1) * P],
                            xst[:, k, :], start=(k == 0), stop=False)
                    for f in range(10):
                        emit_aggT(1)  # last block's transposes, just in time
                        nc.tensor.matmul(
                            ps[:], W3t[:, 4 + f, of * P:(of + 1) * P],
                            aggTt[:, f, :], start=False, stop=(f == 9))
                    nc.scalar.activation(h3Tt[:, of, :], ps[:], RELU,
                                         bias=b3t[:, of:of + 1])
                emit_aggT(100)  # drain any stragglers (non-standard KB)
                oTt = oTp.tile([P, 4, 512], F32R)
                ogs = [ogp.tile([P, FN], F32, name=f"og_{s}_{g}", tag="og")
                       for g in range(4)]
                for of in range(4):
                    ps = mmp2.tile([P, 512], F32)
                    for k in range(5):
                        nc.tensor.matmul(
                            ps[:], W4t[:, k, of * P:(of + 1) * P],
                            h3Tt[:, k, :], start=(k == 0), stop=(k == 4))
                    nc.vector.tensor_scalar_add(oTt[:, of, :], ps[:],
                                                b4t[:, of:of + 1])
                    for g in range(4):
                        pt = ptp.tile([P, P], F32R)
                        nc.tensor.transpose(
                            pt[:], oTt[:, of, g * P:(g + 1) * P], identt[:])
                        nc.vector.tensor_copy(
                            ogs[g][:, of * P:(of + 1) * P],
                            pt[:].bitcast(F32))
                for g in range(4):
                    r0 = s * 512 + g * P
                    nc.sync.dma_start(out_d.ap()[r0:r0 + P, :], ogs[g][:])

    nc.compile()
    return nc


def _prepare(x, row, col, ea):
    """Host-side sharding: sort edges by destination, split nodes into 8
    block-aligned edge-balanced shards, build per-core arrays."""
    N = x.shape[0]
    E = ea.shape[0]
    order = np.argsort(col, kind="stable")
    scol = col[order]
    srow = row[order]
    NBLK = (N + P - 1) // P
    NTOT = NBLK * P

    bounds = [0]
    for p in range(1, NCORES):
        if E > 0:
            t = int(scol[min((p * E) // NCORES, E - 1)])
        else:
            t = (p * NTOT) // NCORES
        b = int(round(t / P)) * P
        b = max(b, bounds[-1] + P)
        b = min(b, NTOT - P * (NCORES - p))
        bounds.append(b)
    bounds.append(NTOT)
    for p in range(1, NCORES + 1):
        assert bounds[p] > bounds[p - 1], f"degenerate shard bounds {bounds}"

    e_split = np.searchsorted(scol, bounds)
    Ec = np.diff(e_split)
    EC = max(4, math.ceil(int(Ec.max()) / P))
    EC = ((EC + 3) // 4) * 4
    EP = EC * P
    nblk = [(bounds[p + 1] - bounds[p]) // P for p in range(NCORES)]
    NB = max(4, ((max(nblk) + 3) // 4) * 4)
    NBP = NB * P
    blkdeg = np.bincount(scol // P, minlength=NBLK)
    KB = max(1, math.ceil(int(blkdeg.max()) / P))

    xpadT = np.zeros((FN, NTOT + NBP), np.float32)
    xpadT[:, :N] = x.T

    cores = []
    for p in range(NCORES):
        s, e = int(e_split[p]), int(e_split[p + 1])
        n0 = bounds[p]
        ne = e - s
        tmp = np.zeros(EP, np.int32)
        tmp[:ne] = srow[s:e]
        rows_t = np.ascontiguousarray(tmp.reshape(EC, P).T)
        eaT = np.zeros((FE, EP), np.float32)
        eaT[:, :ne] = ea[order[s:e]].T
        lcol = (scol[s:e] - n0).astype(np.int64)
        bstart = np.searchsorted(lcol, np.arange(NB + 1) * P)
        gid = np.full((NB, KB, P), 1 << 30, np.int32)
        gid.reshape(NB * KB, P)[:7] = 0
        colb = np.full((NB, KB, P), -1.0, np.float32)
        for b in range(NB):
            sb, eb = int(bstart[b]), int(bstart[b + 1])
            cnt = eb - sb
            assert cnt <= KB * P
            gid[b].reshape(-1)[:cnt] = np.arange(sb, eb, dtype=np.int32)
            colb[b].reshape(-1)[:cnt] = (lcol[sb:eb] - b * P)
        gid_t = np.ascontiguousarray(gid.reshape(NB * KB, P).T)
        colb_t = np.ascontiguousarray(colb.reshape(NB * KB, P).T)
        deg = np.bincount(lcol, minlength=NBP)[:NBP]
        invc_t = np.ascontiguousarray(
            (1.0 / np.maximum(deg, 1.0)).astype(np.float32).reshape(NB, P).T)
        xsT = np.ascontiguousarray(xpadT[:, n0:n0 + NBP])
        cores.append(dict(rows=rows_t, eaT=eaT, gid=gid_t, colb=colb_t,
                          invc=invc_t, xsT=xsT))
    return cores, bounds, EC, NB, KB


def _run(inputs, trace=False):
    x = np.ascontiguousarray(np.asarray(inputs["x"], dtype=np.float32))
    ei = np.asarray(inputs["edge_index"])
    ea = np.ascontiguousarray(np.asarray(inputs["edge_attr"], dtype=np.float32))
    row = ei[0].astype(np.int64)
    col = ei[1].astype(np.int64)
    W1 = np.ascontiguousarray(np.asarray(inputs["W1"], np.float32))
    W2 = np.ascontiguousarray(np.asarray(inputs["W2"], np.float32))
    W3 = np.ascontiguousarray(np.asarray(inputs["W3"], np.float32))
    W4 = np.ascontiguousarray(np.asarray(inputs["W4"], np.float32))
    b1 = np.asarray(inputs["b1"], np.float32)
    b2 = np.asarray(inputs["b2"], np.float32)
    b3 = np.asarray(inputs["b3"], np.float32)
    b4 = np.asarray(inputs["b4"], np.float32)
    N = x.shape[0]

    cores, bounds, EC, NB, KB = _prepare(x, row, col, ea)

    key = (EC, NB, KB, N)
    if key not in _prog_cache:
        _prog_cache[key] = _build(EC, NB, KB, N)
    nc = _prog_cache[key]

    b1t = np.ascontiguousarray(b1.reshape(HID // P, P).T)
    b2t = np.ascontiguousarray(b2.reshape(HID // P, P).T)
    b3t = np.ascontiguousarray(b3.reshape((FN + FE) // P, P).T)
    b4t = np.ascontiguousarray(b4.reshape(FN // P, P).T)
    iota = np.ascontiguousarray(
        np.broadcast_to(np.arange(P, dtype=np.float32), (P, P)))
    ident = np.eye(P, dtype=np.float32)

    in_maps = []
    for p in range(NCORES):
        c = cores[p]
        in_maps.append({
            "x": x, "rows": c["rows"], "eaT": c["eaT"],
            "W1": W1, "W2": W2, "W3": W3, "W4": W4,
            "b1": b1t, "b2": b2t, "b3": b3t, "b4": b4t,
            "gid": c["gid"], "colb": c["colb"], "invc": c["invc"],
            "xsT": c["xsT"], "iota": iota, "ident": ident,
        })

    res = run_bass_kernel_spmd(nc, in_maps, list(range(NCORES)), trace=trace)

    out = np.empty((N, FN), np.float32)
    for p in range(NCORES):
        n0, n1 = bounds[p], min(bounds[p + 1], N)
        if n1 > n0:
            out[n0:n1] = res.results[p]["out"][:n1 - n0]
    return out, res


def kernel(**inputs) -> np.ndarray:
    out, _ = _run(inputs, trace=False)
    return out
